# revision 1
# baseline (speedup 1.0000x reference)
"""nn_MergeWindows — Trainium2 Bass kernel (8 NeuronCores, SPMD over image rows).

Key observation: the reference's sequential merge scan over candidate channel
pairs depends only on tiny metadata — per-channel edge-touch bits along the
window boundaries (rows/cols 511/512 of the 1024x1024 image) and cosine sims
of the [4,7,64] slot features.  The final output is exactly

    out[b, c, y, x] = 1.0  iff  remap[argmax_d masks[b, d, y, x]] == c

where remap: [32]->[32] merges channels per the scan.  remap is computed on
the host (numpy, microseconds — it reads 4 boundary strips), and the heavy
per-pixel work (argmax over 32 channels + relabel + one-hot, 128 MiB in /
128 MiB out) runs on 8 NeuronCores, each handling 128 of the 1024 rows.

Device pipeline per [128 rows, 32 ch, 128 cols] tile (pixel-major layout,
rows on partitions), spread across three engines:
  1. mx   = reduce_max over channels                      (DVE, strided AP)
  2. u    = masks - mx        (winner becomes exactly 0)  (DVE, bcast AP)
  3. v    = u*2^50 + K[d],   K[d] = remap[d]+1+64*(32-d)  (ACT, 32 Identity
     channel slices: scale=2^50, bias=kfull[:, d])
  4. s    = reduce_max over channels of v = K[winner]     (DVE)
     -> first-match on ties, like argmax: the 64*(32-d) term dominates
  5. si   = (int32)s & 63 = remap[winner]+1               (Pool cast + DVE and)
  6. out[:, c, :] = is_equal(si, c+1)                     (Pool, 32 int TS
     channel slices)

(tensor_tensor_reduce, GPSIMD tensor_tensor, and the mod ALU op are rejected
by the pinned neuronxcc; only one sync-wait per instruction is allowed, hence
the wait-split post-pass below.)
"""

import json

import numpy as np

N_WINDOWS = 4
WIN_H = WIN_W = 512
IMG_H = IMG_W = 1024
C = 32
MPW = C // N_WINDOWS
SLOT_DIM = 64
SIM_THRESH = 0.1

N_CORES = 8
ROWS_PER_CORE = IMG_H // N_CORES  # 128
G = 128          # column-tile width
NTILES = IMG_W // G
POOL_BUFS = (3, 2, 3)        # (inp, work, outp) tile-pool buffer counts

_cache = {}


# --------------------------------------------------------------------------
# host-side merge decision (mirrors reference._merge_windows metadata math)
# --------------------------------------------------------------------------
def _compute_remap(masks, slot_features, pl, pt):
    B, Ch, H, W = masks.shape
    mpw = Ch // N_WINDOWS
    ranges = [(i * mpw, (i + 1) * mpw) for i in range(N_WINDOWS)]

    adjacency = []
    for i in range(N_WINDOWS):
        for j in range(i + 1, N_WINDOWS):
            if pt[i] == pt[j] and abs(pl[i] - pl[j]) == WIN_W:
                adjacency.append((i, j, True) if pl[i] < pl[j] else (j, i, True))
            if pl[i] == pl[j] and abs(pt[i] - pt[j]) == WIN_H:
                adjacency.append((i, j, False) if pt[i] < pt[j] else (j, i, False))

    edge_l = np.zeros(Ch, bool)
    edge_r = np.zeros(Ch, bool)
    edge_t = np.zeros(Ch, bool)
    edge_b = np.zeros(Ch, bool)
    m0 = masks[0]
    for wi, (s, e) in enumerate(ranges):
        ys, ye = max(pt[wi], 0), min(pt[wi] + WIN_H, H)
        xs, xe = max(pl[wi], 0), min(pl[wi] + WIN_W, W)
        if ys >= ye or xs >= xe:
            continue
        ids_l = np.argmax(m0[:, ys:ye, xs], axis=0)
        ids_r = np.argmax(m0[:, ys:ye, xe - 1], axis=0)
        ids_t = np.argmax(m0[:, ys, xs:xe], axis=0)
        ids_b = np.argmax(m0[:, ye - 1, xs:xe], axis=0)
        for k in range(s, e):
            edge_l[k] = np.any(ids_l == k)
            edge_r[k] = np.any(ids_r == k)
            edge_t[k] = np.any(ids_t == k)
            edge_b[k] = np.any(ids_b == k)

    ci_l, cj_l, wi_l, wj_l, hz_l = [], [], [], [], []
    for wi, wj, horiz in adjacency:
        si, ei = ranges[wi]
        sj, ej = ranges[wj]
        for ci in range(si + 1, ei):
            for cj in range(sj + 1, ej):
                ci_l.append(ci)
                cj_l.append(cj)
                wi_l.append(wi)
                wj_l.append(wj)
                hz_l.append(horiz)

    target = np.arange(Ch)
    if not ci_l:
        return target

    sf = np.asarray(slot_features, np.float32)
    sf_n = sf / (np.linalg.norm(sf, axis=-1, keepdims=True) + np.float32(1e-8))
    ci_a = np.array(ci_l)
    cj_a = np.array(cj_l)
    rel_i = ci_a % mpw - 1
    rel_j = cj_a % mpw - 1
    fi = sf_n[np.array(wi_l), rel_i]
    fj = sf_n[np.array(wj_l), rel_j]
    sims = np.sum(fi * fj, axis=-1)
    hz = np.array(hz_l)
    edge_ok = np.where(hz, edge_r[ci_a] & edge_l[cj_a], edge_b[ci_a] & edge_t[cj_a])
    passing = edge_ok & (sims > np.float32(SIM_THRESH))

    merged = np.zeros(Ch, bool)
    for ci, cj, ok in zip(ci_l, cj_l, passing):
        if ok and not merged[ci] and not merged[cj]:
            keep, rem = min(ci, cj), max(ci, cj)
            target[target == rem] = keep
            merged[rem] = True
    return target


# --------------------------------------------------------------------------
# wait-split post-pass: the pinned neuronxcc allows only ONE sync wait per
# instruction; hoist extras onto preceding same-engine EventSemaphore insts.
# --------------------------------------------------------------------------
def _split_excess_waits(bir_json_bytes, limit=1):
    j = json.loads(bir_json_bytes)
    counter = [0]
    for fn in j.get("functions", []):
        for bb in fn.get("blocks", []):
            new_insts = []
            for inst in bb.get("instructions", []):
                si = inst.get("sync_info") or {}
                waits = si.get("on_wait") or []
                if len(waits) > limit:
                    extra = waits[: len(waits) - limit]
                    si["on_wait"] = waits[len(waits) - limit:]
                    inst["sync_info"] = si
                    for i in range(0, len(extra), limit):
                        counter[0] += 1
                        new_insts.append({
                            "engine": inst["engine"],
                            "ins": [],
                            "name": f"{inst['name']}_hoistw{counter[0]}",
                            "opcode": "EventSemaphore",
                            "outs": [],
                            "sync_info": {"on_update": [],
                                          "on_wait": extra[i: i + limit]},
                        })
                new_insts.append(inst)
            bb["instructions"] = new_insts
    return json.dumps(j).encode()


def _build_program():
    if "nc" in _cache:
        return _cache["nc"]

    import concourse.bass as bass
    import concourse.tile as tile
    from concourse import mybir

    f32 = mybir.dt.float32
    nc = bass.Bass()
    masks_in = nc.dram_tensor("masks", [C, ROWS_PER_CORE, IMG_W], f32,
                              kind="ExternalInput")
    i32 = mybir.dt.int32
    kfull_in = nc.dram_tensor("kfull", [128, C], f32, kind="ExternalInput")
    out_dram = nc.dram_tensor("out", [C, ROWS_PER_CORE, IMG_W], f32,
                              kind="ExternalOutput")

    with tile.TileContext(nc) as tc:
        with (
            tc.tile_pool(name="inp", bufs=POOL_BUFS[0]) as inp,
            tc.tile_pool(name="work", bufs=POOL_BUFS[1]) as work,
            tc.tile_pool(name="outp", bufs=POOL_BUFS[2]) as outp,
            tc.tile_pool(name="small", bufs=4) as small,
            tc.tile_pool(name="singles", bufs=1) as singles,
        ):
            kfull = singles.tile([128, C], f32)
            nc.sync.dma_start(kfull[:], kfull_in[:])

            for t in range(NTILES):
                sl = slice(G * t, G * (t + 1))
                in_tile = inp.tile([128, C, G], f32, tag="in_tile")
                nc.sync.dma_start(
                    in_tile[:], masks_in[:, :, sl].rearrange("d p g -> p d g"))

                mx = small.tile([128, G], f32, tag="mx")
                nc.vector.tensor_reduce(
                    out=mx[:], in_=in_tile[:].rearrange("p d g -> p g d"),
                    axis=mybir.AxisListType.X, op=mybir.AluOpType.max)

                u = work.tile([128, C, G], f32, tag="u")
                mx_ap = mx[:]
                mx_b = bass.AP(tensor=mx_ap.tensor, offset=mx_ap.offset,
                               ap=[mx_ap.ap[0], [0, C], mx_ap.ap[-1]])
                nc.vector.tensor_tensor(out=u[:], in0=in_tile[:], in1=mx_b,
                                        op=mybir.AluOpType.subtract)

                for d in range(C):
                    nc.scalar.activation(
                        u[:, d, :], u[:, d, :],
                        mybir.ActivationFunctionType.Identity,
                        bias=kfull[:, d:d + 1], scale=float(2.0 ** 50))

                s = small.tile([128, G], f32, tag="s")
                nc.vector.tensor_reduce(
                    out=s[:], in_=u[:].rearrange("p d g -> p g d"),
                    axis=mybir.AxisListType.X, op=mybir.AluOpType.max)
                si = small.tile([128, G], i32, tag="si")
                nc.gpsimd.tensor_copy(si[:], s[:])
                nc.vector.tensor_scalar(out=si[:], in0=si[:],
                                        scalar1=63, scalar2=None,
                                        op0=mybir.AluOpType.bitwise_and)

                out_tile = outp.tile([128, C, G], f32, tag="out_tile")
                for c in range(C):
                    nc.gpsimd.tensor_scalar(out=out_tile[:, c, :], in0=si[:],
                                            scalar1=c + 1, scalar2=None,
                                            op0=mybir.AluOpType.is_equal)

                nc.sync.dma_start(
                    out_dram[:, :, sl].rearrange("c p g -> p c g"), out_tile[:])

    orig = nc.to_json_bytes
    nc.to_json_bytes = lambda: _split_excess_waits(orig())
    _cache["nc"] = nc
    return nc


def kernel(masks, slot_features, pad_left, pad_top):
    from concourse.bass_utils import run_bass_kernel_spmd

    masks = np.asarray(masks, np.float32)
    slot_features = np.asarray(slot_features, np.float32)
    pl = [int(v) for v in np.asarray(pad_left)]
    pt = [int(v) for v in np.asarray(pad_top)]

    remap = _compute_remap(masks, slot_features, pl, pt)

    K = (remap + 1 + 64.0 * (C - np.arange(C))).astype(np.float32)
    kfull = np.ascontiguousarray(np.tile(K[None, :], (128, 1)))

    nc = _build_program()
    in_maps = []
    for i in range(N_CORES):
        slab = np.ascontiguousarray(
            masks[0, :, i * ROWS_PER_CORE:(i + 1) * ROWS_PER_CORE, :])
        in_maps.append({"masks": slab, "kfull": kfull})

    res = run_bass_kernel_spmd(nc, in_maps, core_ids=list(range(N_CORES)))

    out = np.empty((1, C, IMG_H, IMG_W), np.float32)
    for i, r in enumerate(res.results):
        out[0, :, i * ROWS_PER_CORE:(i + 1) * ROWS_PER_CORE, :] = r["out"]
    return out



# revision 5
# speedup vs baseline: 5.0829x; 5.0829x over previous
"""nn_MergeWindows — Trainium2 Bass kernel (8 NeuronCores, SPMD over image rows).

The reference's output is out[b, c, y, x] = 1.0 iff remap[argmax_d masks[b, d,
y, x]] == c, where remap: [32]->[32] is the channel-merge map decided by the
sequential scan.  remap depends only on tiny metadata (boundary-strip argmaxes
+ slot-feature cosine sims) and is computed on the host in microseconds.

Device pipeline per [128 rows, 32 ch, G cols] tile, everything on the DVE:
  1. 5-level contiguous max-tree over the channel dim -> mx [128, G]
  2. win = is_equal(in, mx_broadcast)  -> one-hot over channels, bf16
  3. merge fixups: win[:, keep] += win[:, rem]; win[:, rem] = 0
     (a dozen tiny [128, G] ops, baked from remap)
  4. DMA the bf16 one-hot out; the host casts to f32 (0/1 exact in bf16).

Tied maxima (in[c] == in[d] == mx) make is_equal fire twice where argmax picks
the first channel.  Those pixels are exactly where the output's channel-sum is
not 1.0, so the host detects them from the assembled output and patches them
from the input directly (a handful of pixels on real data).
"""

import json

import numpy as np

N_WINDOWS = 4
WIN_H = WIN_W = 512
IMG_H = IMG_W = 1024
C = 32
MPW = C // N_WINDOWS
SLOT_DIM = 64
SIM_THRESH = 0.1

N_CORES = 8
ROWS_PER_CORE = IMG_H // N_CORES  # 128
G = 256                           # column-tile width
NTILES = IMG_W // G

_cache = {}


# --------------------------------------------------------------------------
# host-side merge decision (mirrors reference._merge_windows metadata math)
# --------------------------------------------------------------------------
def _compute_remap(masks, slot_features, pl, pt):
    B, Ch, H, W = masks.shape
    mpw = Ch // N_WINDOWS
    ranges = [(i * mpw, (i + 1) * mpw) for i in range(N_WINDOWS)]

    adjacency = []
    for i in range(N_WINDOWS):
        for j in range(i + 1, N_WINDOWS):
            if pt[i] == pt[j] and abs(pl[i] - pl[j]) == WIN_W:
                adjacency.append((i, j, True) if pl[i] < pl[j] else (j, i, True))
            if pl[i] == pl[j] and abs(pt[i] - pt[j]) == WIN_H:
                adjacency.append((i, j, False) if pt[i] < pt[j] else (j, i, False))

    edge_l = np.zeros(Ch, bool)
    edge_r = np.zeros(Ch, bool)
    edge_t = np.zeros(Ch, bool)
    edge_b = np.zeros(Ch, bool)
    m0 = masks[0]
    for wi, (s, e) in enumerate(ranges):
        ys, ye = max(pt[wi], 0), min(pt[wi] + WIN_H, H)
        xs, xe = max(pl[wi], 0), min(pl[wi] + WIN_W, W)
        if ys >= ye or xs >= xe:
            continue
        ids_l = np.argmax(m0[:, ys:ye, xs], axis=0)
        ids_r = np.argmax(m0[:, ys:ye, xe - 1], axis=0)
        ids_t = np.argmax(m0[:, ys, xs:xe], axis=0)
        ids_b = np.argmax(m0[:, ye - 1, xs:xe], axis=0)
        for k in range(s, e):
            edge_l[k] = np.any(ids_l == k)
            edge_r[k] = np.any(ids_r == k)
            edge_t[k] = np.any(ids_t == k)
            edge_b[k] = np.any(ids_b == k)

    ci_l, cj_l, wi_l, wj_l, hz_l = [], [], [], [], []
    for wi, wj, horiz in adjacency:
        si, ei = ranges[wi]
        sj, ej = ranges[wj]
        for ci in range(si + 1, ei):
            for cj in range(sj + 1, ej):
                ci_l.append(ci)
                cj_l.append(cj)
                wi_l.append(wi)
                wj_l.append(wj)
                hz_l.append(horiz)

    target = np.arange(Ch)
    if not ci_l:
        return target

    sf = np.asarray(slot_features, np.float32)
    sf_n = sf / (np.linalg.norm(sf, axis=-1, keepdims=True) + np.float32(1e-8))
    ci_a = np.array(ci_l)
    cj_a = np.array(cj_l)
    rel_i = ci_a % mpw - 1
    rel_j = cj_a % mpw - 1
    fi = sf_n[np.array(wi_l), rel_i]
    fj = sf_n[np.array(wj_l), rel_j]
    sims = np.sum(fi * fj, axis=-1)
    hz = np.array(hz_l)
    edge_ok = np.where(hz, edge_r[ci_a] & edge_l[cj_a], edge_b[ci_a] & edge_t[cj_a])
    passing = edge_ok & (sims > np.float32(SIM_THRESH))

    merged = np.zeros(Ch, bool)
    for ci, cj, ok in zip(ci_l, cj_l, passing):
        if ok and not merged[ci] and not merged[cj]:
            keep, rem = min(ci, cj), max(ci, cj)
            target[target == rem] = keep
            merged[rem] = True
    return target


def _fixup_ops(remap):
    """Grouped (rem, keep, len) add runs and (rem, len) zero runs."""
    pairs = [(r, int(remap[r])) for r in range(C) if remap[r] != r]
    adds, zeros = [], []
    for r, k in pairs:
        if adds and adds[-1][0] + adds[-1][2] == r and adds[-1][1] + adds[-1][2] == k:
            adds[-1][2] += 1
        else:
            adds.append([r, k, 1])
    for r, _ in pairs:
        if zeros and zeros[-1][0] + zeros[-1][1] == r:
            zeros[-1][1] += 1
        else:
            zeros.append([r, 1])
    return adds, zeros


# --------------------------------------------------------------------------
# wait-split post-pass: the pinned neuronxcc allows only ONE sync wait per
# instruction; hoist extras onto preceding same-engine EventSemaphore insts.
# --------------------------------------------------------------------------
def _split_excess_waits(bir_json_bytes, limit=1):
    j = json.loads(bir_json_bytes)
    counter = [0]
    for fn in j.get("functions", []):
        for bb in fn.get("blocks", []):
            new_insts = []
            for inst in bb.get("instructions", []):
                si = inst.get("sync_info") or {}
                waits = si.get("on_wait") or []
                if len(waits) > limit:
                    extra = waits[: len(waits) - limit]
                    si["on_wait"] = waits[len(waits) - limit:]
                    inst["sync_info"] = si
                    for i in range(0, len(extra), limit):
                        counter[0] += 1
                        new_insts.append({
                            "engine": inst["engine"],
                            "ins": [],
                            "name": f"{inst['name']}_hoistw{counter[0]}",
                            "opcode": "EventSemaphore",
                            "outs": [],
                            "sync_info": {"on_update": [],
                                          "on_wait": extra[i: i + limit]},
                        })
                new_insts.append(inst)
            bb["instructions"] = new_insts
    return json.dumps(j).encode()


def _build_program(remap):
    key = tuple(int(v) for v in remap)
    if key in _cache:
        return _cache[key]

    import concourse.bass as bass
    import concourse.tile as tile
    from concourse import mybir

    f32 = mybir.dt.float32
    bf16 = mybir.dt.bfloat16
    adds, zeros = _fixup_ops(remap)

    nc = bass.Bass()
    masks_in = nc.dram_tensor("masks", [C, ROWS_PER_CORE, IMG_W], f32,
                              kind="ExternalInput")
    out_dram = nc.dram_tensor("out", [C, ROWS_PER_CORE, IMG_W], bf16,
                              kind="ExternalOutput")

    with tile.TileContext(nc) as tc:
        with (
            tc.tile_pool(name="inp", bufs=2) as inp,
            tc.tile_pool(name="outp", bufs=3) as outp,
            tc.tile_pool(name="tree", bufs=1) as tree,
            tc.tile_pool(name="small", bufs=2) as small,
        ):
            for t in range(NTILES):
                sl = slice(G * t, G * (t + 1))
                in_tile = inp.tile([128, C, G], f32, tag="in_tile")
                nc.sync.dma_start(
                    in_tile[:], masks_in[:, :, sl].rearrange("d p g -> p d g"))

                # contiguous max-tree over the channel dim
                tt = tree.tile([128, 16, G], f32, tag="tt")
                nc.vector.tensor_tensor(
                    out=tt[:], in0=in_tile[:, 0:16, :], in1=in_tile[:, 16:32, :],
                    op=mybir.AluOpType.max)
                for h in (8, 4, 2):
                    nc.vector.tensor_tensor(
                        out=tt[:, 0:h, :], in0=tt[:, 0:h, :], in1=tt[:, h:2 * h, :],
                        op=mybir.AluOpType.max)
                mx = small.tile([128, G], f32, tag="mx")
                nc.vector.tensor_tensor(
                    out=mx[:], in0=tt[:, 0, :], in1=tt[:, 1, :],
                    op=mybir.AluOpType.max)

                # one-hot: win[p, c, g] = (in[p, c, g] == mx[p, g])
                mx_ap = mx[:]
                mx_b = bass.AP(tensor=mx_ap.tensor, offset=mx_ap.offset,
                               ap=[mx_ap.ap[0], [0, C], mx_ap.ap[-1]])
                out_tile = outp.tile([128, C, G], bf16, tag="out_tile")
                nc.vector.tensor_tensor(out=out_tile[:], in0=in_tile[:],
                                        in1=mx_b, op=mybir.AluOpType.is_equal)

                # channel merges baked from remap
                for r, k, n in adds:
                    nc.vector.tensor_tensor(
                        out=out_tile[:, k:k + n, :], in0=out_tile[:, k:k + n, :],
                        in1=out_tile[:, r:r + n, :], op=mybir.AluOpType.add)
                for r, n in zeros:
                    nc.vector.memset(out_tile[:, r:r + n, :], 0.0)

                nc.sync.dma_start(
                    out_dram[:, :, sl].rearrange("c p g -> p c g"), out_tile[:])

    orig = nc.to_json_bytes
    nc.to_json_bytes = lambda: _split_excess_waits(orig())
    _cache[key] = nc
    return nc


def kernel(masks, slot_features, pad_left, pad_top):
    from concourse.bass_utils import run_bass_kernel_spmd

    masks = np.asarray(masks, np.float32)
    slot_features = np.asarray(slot_features, np.float32)
    pl = [int(v) for v in np.asarray(pad_left)]
    pt = [int(v) for v in np.asarray(pad_top)]

    remap = _compute_remap(masks, slot_features, pl, pt)
    nc = _build_program(remap)

    in_maps = []
    for i in range(N_CORES):
        slab = np.ascontiguousarray(
            masks[0, :, i * ROWS_PER_CORE:(i + 1) * ROWS_PER_CORE, :])
        in_maps.append({"masks": slab})

    res = run_bass_kernel_spmd(nc, in_maps, core_ids=list(range(N_CORES)))

    out = np.empty((1, C, IMG_H, IMG_W), np.float32)
    for i, r in enumerate(res.results):
        out[0, :, i * ROWS_PER_CORE:(i + 1) * ROWS_PER_CORE, :] = (
            np.asarray(r["out"]).astype(np.float32))

    # patch tied-max pixels (channel-sum != 1) from the input directly
    s = out[0].sum(axis=0)
    ys, xs = np.nonzero(s != 1.0)
    for y, x in zip(ys, xs):
        d = int(np.argmax(masks[0, :, y, x]))
        out[0, :, y, x] = 0.0
        out[0, remap[d], y, x] = 1.0
    return out


# revision 8
# speedup vs baseline: 5.2438x; 1.0317x over previous
"""nn_MergeWindows — Trainium2 Bass kernel (8 NeuronCores, SPMD over image rows).

The reference's output is out[b, c, y, x] = 1.0 iff remap[argmax_d masks[b, d,
y, x]] == c, where remap: [32]->[32] is the channel-merge map decided by the
sequential scan.  remap depends only on tiny metadata (boundary-strip argmaxes
+ slot-feature cosine sims) and is computed on the host in microseconds.

Device pipeline per [128 rows, 32 ch, G cols] tile, everything on the DVE:
  1. 5-level contiguous max-tree over the channel dim -> mx [128, G]
  2. win = is_equal(in, mx_broadcast)  -> one-hot over channels, bf16
  3. merge fixups: win[:, keep] += win[:, rem]; win[:, rem] = 0
     (a dozen tiny [128, G] ops, baked from remap)
  4. DMA the bf16 one-hot out; the host casts to f32 (0/1 exact in bf16).

Tied maxima (in[c] == in[d] == mx) make is_equal fire twice where argmax picks
the first channel.  Those pixels are exactly where the output's channel-sum is
not 1.0, so the host detects them from the assembled output and patches them
from the input directly (a handful of pixels on real data).
"""

import json

import numpy as np

N_WINDOWS = 4
WIN_H = WIN_W = 512
IMG_H = IMG_W = 1024
C = 32
MPW = C // N_WINDOWS
SLOT_DIM = 64
SIM_THRESH = 0.1

N_CORES = 8
ROWS_PER_CORE = IMG_H // N_CORES  # 128
G = 512                           # column-tile width
NTILES = IMG_W // G

_cache = {}


# --------------------------------------------------------------------------
# host-side merge decision (mirrors reference._merge_windows metadata math)
# --------------------------------------------------------------------------
def _compute_remap(masks, slot_features, pl, pt):
    B, Ch, H, W = masks.shape
    mpw = Ch // N_WINDOWS
    ranges = [(i * mpw, (i + 1) * mpw) for i in range(N_WINDOWS)]

    adjacency = []
    for i in range(N_WINDOWS):
        for j in range(i + 1, N_WINDOWS):
            if pt[i] == pt[j] and abs(pl[i] - pl[j]) == WIN_W:
                adjacency.append((i, j, True) if pl[i] < pl[j] else (j, i, True))
            if pl[i] == pl[j] and abs(pt[i] - pt[j]) == WIN_H:
                adjacency.append((i, j, False) if pt[i] < pt[j] else (j, i, False))

    edge_l = np.zeros(Ch, bool)
    edge_r = np.zeros(Ch, bool)
    edge_t = np.zeros(Ch, bool)
    edge_b = np.zeros(Ch, bool)
    m0 = masks[0]
    for wi, (s, e) in enumerate(ranges):
        ys, ye = max(pt[wi], 0), min(pt[wi] + WIN_H, H)
        xs, xe = max(pl[wi], 0), min(pl[wi] + WIN_W, W)
        if ys >= ye or xs >= xe:
            continue
        ids_l = np.argmax(m0[:, ys:ye, xs], axis=0)
        ids_r = np.argmax(m0[:, ys:ye, xe - 1], axis=0)
        ids_t = np.argmax(m0[:, ys, xs:xe], axis=0)
        ids_b = np.argmax(m0[:, ye - 1, xs:xe], axis=0)
        for k in range(s, e):
            edge_l[k] = np.any(ids_l == k)
            edge_r[k] = np.any(ids_r == k)
            edge_t[k] = np.any(ids_t == k)
            edge_b[k] = np.any(ids_b == k)

    ci_l, cj_l, wi_l, wj_l, hz_l = [], [], [], [], []
    for wi, wj, horiz in adjacency:
        si, ei = ranges[wi]
        sj, ej = ranges[wj]
        for ci in range(si + 1, ei):
            for cj in range(sj + 1, ej):
                ci_l.append(ci)
                cj_l.append(cj)
                wi_l.append(wi)
                wj_l.append(wj)
                hz_l.append(horiz)

    target = np.arange(Ch)
    if not ci_l:
        return target

    sf = np.asarray(slot_features, np.float32)
    sf_n = sf / (np.linalg.norm(sf, axis=-1, keepdims=True) + np.float32(1e-8))
    ci_a = np.array(ci_l)
    cj_a = np.array(cj_l)
    rel_i = ci_a % mpw - 1
    rel_j = cj_a % mpw - 1
    fi = sf_n[np.array(wi_l), rel_i]
    fj = sf_n[np.array(wj_l), rel_j]
    sims = np.sum(fi * fj, axis=-1)
    hz = np.array(hz_l)
    edge_ok = np.where(hz, edge_r[ci_a] & edge_l[cj_a], edge_b[ci_a] & edge_t[cj_a])
    passing = edge_ok & (sims > np.float32(SIM_THRESH))

    merged = np.zeros(Ch, bool)
    for ci, cj, ok in zip(ci_l, cj_l, passing):
        if ok and not merged[ci] and not merged[cj]:
            keep, rem = min(ci, cj), max(ci, cj)
            target[target == rem] = keep
            merged[rem] = True
    return target


def _fixup_ops(remap):
    """Grouped (rem, keep, len) add runs and (rem, len) zero runs."""
    pairs = [(r, int(remap[r])) for r in range(C) if remap[r] != r]
    adds, zeros = [], []
    for r, k in pairs:
        if adds and adds[-1][0] + adds[-1][2] == r and adds[-1][1] + adds[-1][2] == k:
            adds[-1][2] += 1
        else:
            adds.append([r, k, 1])
    for r, _ in pairs:
        if zeros and zeros[-1][0] + zeros[-1][1] == r:
            zeros[-1][1] += 1
        else:
            zeros.append([r, 1])
    return adds, zeros


# --------------------------------------------------------------------------
# wait-split post-pass: the pinned neuronxcc allows only ONE sync wait per
# instruction; hoist extras onto preceding same-engine EventSemaphore insts.
# --------------------------------------------------------------------------
def _split_excess_waits(bir_json_bytes, limit=1):
    j = json.loads(bir_json_bytes)
    counter = [0]
    for fn in j.get("functions", []):
        for bb in fn.get("blocks", []):
            new_insts = []
            for inst in bb.get("instructions", []):
                si = inst.get("sync_info") or {}
                waits = si.get("on_wait") or []
                if len(waits) > limit:
                    extra = waits[: len(waits) - limit]
                    si["on_wait"] = waits[len(waits) - limit:]
                    inst["sync_info"] = si
                    for i in range(0, len(extra), limit):
                        counter[0] += 1
                        new_insts.append({
                            "engine": inst["engine"],
                            "ins": [],
                            "name": f"{inst['name']}_hoistw{counter[0]}",
                            "opcode": "EventSemaphore",
                            "outs": [],
                            "sync_info": {"on_update": [],
                                          "on_wait": extra[i: i + limit]},
                        })
                new_insts.append(inst)
            bb["instructions"] = new_insts
    return json.dumps(j).encode()


def _build_program(remap):
    key = tuple(int(v) for v in remap)
    if key in _cache:
        return _cache[key]

    import concourse.bass as bass
    import concourse.tile as tile
    from concourse import mybir

    f32 = mybir.dt.float32
    bf16 = mybir.dt.bfloat16
    adds, zeros = _fixup_ops(remap)

    nc = bass.Bass()
    masks_in = nc.dram_tensor("masks", [C, ROWS_PER_CORE, IMG_W], f32,
                              kind="ExternalInput")
    out_dram = nc.dram_tensor("out", [C, ROWS_PER_CORE, IMG_W], bf16,
                              kind="ExternalOutput")

    with tile.TileContext(nc) as tc:
        with (
            tc.tile_pool(name="inp", bufs=2) as inp,
            tc.tile_pool(name="outp", bufs=2) as outp,
            tc.tile_pool(name="tree", bufs=1) as tree,
            tc.tile_pool(name="small", bufs=2) as small,
        ):
            for t in range(NTILES):
                sl = slice(G * t, G * (t + 1))
                in_tile = inp.tile([128, C, G], f32, tag="in_tile")
                # channel-group split so the max-fold can start before the
                # whole tile lands
                for c0 in range(0, C, 8):
                    nc.sync.dma_start(
                        in_tile[:, c0:c0 + 8, :],
                        masks_in[c0:c0 + 8, :, sl].rearrange("d p g -> p d g"))

                # contiguous max-fold over the channel dim into a 4-ch strip
                tt = tree.tile([128, 4, G], f32, tag="tt")
                nc.vector.tensor_tensor(
                    out=tt[:], in0=in_tile[:, 0:4, :], in1=in_tile[:, 4:8, :],
                    op=mybir.AluOpType.max)
                for c0 in range(8, C, 4):
                    nc.vector.tensor_tensor(
                        out=tt[:], in0=tt[:], in1=in_tile[:, c0:c0 + 4, :],
                        op=mybir.AluOpType.max)
                nc.vector.tensor_tensor(
                    out=tt[:, 0:2, :], in0=tt[:, 0:2, :], in1=tt[:, 2:4, :],
                    op=mybir.AluOpType.max)
                mx = small.tile([128, G], f32, tag="mx")
                nc.vector.tensor_tensor(
                    out=mx[:], in0=tt[:, 0, :], in1=tt[:, 1, :],
                    op=mybir.AluOpType.max)

                # one-hot: win[p, c, g] = (in[p, c, g] == mx[p, g])
                mx_ap = mx[:]
                mx_b = bass.AP(tensor=mx_ap.tensor, offset=mx_ap.offset,
                               ap=[mx_ap.ap[0], [0, C], mx_ap.ap[-1]])
                out_tile = outp.tile([128, C, G], bf16, tag="out_tile")
                nc.vector.tensor_tensor(out=out_tile[:], in0=in_tile[:],
                                        in1=mx_b, op=mybir.AluOpType.is_equal)

                # channel merges baked from remap
                for r, k, n in adds:
                    nc.vector.tensor_tensor(
                        out=out_tile[:, k:k + n, :], in0=out_tile[:, k:k + n, :],
                        in1=out_tile[:, r:r + n, :], op=mybir.AluOpType.add)
                for r, n in zeros:
                    nc.gpsimd.memset(out_tile[:, r:r + n, :], 0.0)

                nc.scalar.dma_start(
                    out_dram[:, :, sl].rearrange("c p g -> p c g"), out_tile[:])

    orig = nc.to_json_bytes
    nc.to_json_bytes = lambda: _split_excess_waits(orig())
    _cache[key] = nc
    return nc


def kernel(masks, slot_features, pad_left, pad_top):
    from concourse.bass_utils import run_bass_kernel_spmd

    masks = np.asarray(masks, np.float32)
    slot_features = np.asarray(slot_features, np.float32)
    pl = [int(v) for v in np.asarray(pad_left)]
    pt = [int(v) for v in np.asarray(pad_top)]

    remap = _compute_remap(masks, slot_features, pl, pt)
    nc = _build_program(remap)

    in_maps = []
    for i in range(N_CORES):
        slab = np.ascontiguousarray(
            masks[0, :, i * ROWS_PER_CORE:(i + 1) * ROWS_PER_CORE, :])
        in_maps.append({"masks": slab})

    res = run_bass_kernel_spmd(nc, in_maps, core_ids=list(range(N_CORES)))

    out = np.empty((1, C, IMG_H, IMG_W), np.float32)
    for i, r in enumerate(res.results):
        out[0, :, i * ROWS_PER_CORE:(i + 1) * ROWS_PER_CORE, :] = (
            np.asarray(r["out"]).astype(np.float32))

    # patch tied-max pixels (channel-sum != 1) from the input directly
    s = out[0].sum(axis=0)
    ys, xs = np.nonzero(s != 1.0)
    for y, x in zip(ys, xs):
        d = int(np.argmax(masks[0, :, y, x]))
        out[0, :, y, x] = 0.0
        out[0, remap[d], y, x] = 1.0
    return out


# revision 12
# speedup vs baseline: 6.5189x; 1.2432x over previous
"""nn_MergeWindows — Trainium2 Bass kernel (8 NeuronCores, SPMD over image rows).

The reference's output is out[b, c, y, x] = 1.0 iff remap[argmax_d masks[b, d,
y, x]] == c, where remap: [32]->[32] is the channel-merge map decided by the
sequential scan.  remap depends only on tiny metadata (boundary-strip argmaxes
+ slot-feature cosine sims) and is computed on the host in microseconds.

Device pipeline per [128 rows, 32 ch, G cols] tile, everything on the DVE:
  1. 5-level contiguous max-tree over the channel dim -> mx [128, G]
  2. win = is_equal(in, mx_broadcast)  -> one-hot over channels, bf16
  3. merge fixups: win[:, keep] += win[:, rem]; win[:, rem] = 0
     (a dozen tiny [128, G] ops, baked from remap)
  4. DMA the bf16 one-hot out; the host casts to f32 (0/1 exact in bf16).

Tied maxima (in[c] == in[d] == mx) make is_equal fire twice where argmax picks
the first channel.  Those pixels are exactly where the output's channel-sum is
not 1.0, so the host detects them from the assembled output and patches them
from the input directly (a handful of pixels on real data).
"""

import json

import numpy as np

N_WINDOWS = 4
WIN_H = WIN_W = 512
IMG_H = IMG_W = 1024
C = 32
MPW = C // N_WINDOWS
SLOT_DIM = 64
SIM_THRESH = 0.1

N_CORES = 8
ROWS_PER_CORE = IMG_H // N_CORES  # 128
G = 512                           # column-tile width
NTILES = IMG_W // G

_cache = {}


# --------------------------------------------------------------------------
# host-side merge decision (mirrors reference._merge_windows metadata math)
# --------------------------------------------------------------------------
def _compute_remap(masks, slot_features, pl, pt):
    B, Ch, H, W = masks.shape
    mpw = Ch // N_WINDOWS
    ranges = [(i * mpw, (i + 1) * mpw) for i in range(N_WINDOWS)]

    adjacency = []
    for i in range(N_WINDOWS):
        for j in range(i + 1, N_WINDOWS):
            if pt[i] == pt[j] and abs(pl[i] - pl[j]) == WIN_W:
                adjacency.append((i, j, True) if pl[i] < pl[j] else (j, i, True))
            if pl[i] == pl[j] and abs(pt[i] - pt[j]) == WIN_H:
                adjacency.append((i, j, False) if pt[i] < pt[j] else (j, i, False))

    edge_l = np.zeros(Ch, bool)
    edge_r = np.zeros(Ch, bool)
    edge_t = np.zeros(Ch, bool)
    edge_b = np.zeros(Ch, bool)
    m0 = masks[0]
    for wi, (s, e) in enumerate(ranges):
        ys, ye = max(pt[wi], 0), min(pt[wi] + WIN_H, H)
        xs, xe = max(pl[wi], 0), min(pl[wi] + WIN_W, W)
        if ys >= ye or xs >= xe:
            continue
        ids_l = np.argmax(m0[:, ys:ye, xs], axis=0)
        ids_r = np.argmax(m0[:, ys:ye, xe - 1], axis=0)
        ids_t = np.argmax(m0[:, ys, xs:xe], axis=0)
        ids_b = np.argmax(m0[:, ye - 1, xs:xe], axis=0)
        for k in range(s, e):
            edge_l[k] = np.any(ids_l == k)
            edge_r[k] = np.any(ids_r == k)
            edge_t[k] = np.any(ids_t == k)
            edge_b[k] = np.any(ids_b == k)

    ci_l, cj_l, wi_l, wj_l, hz_l = [], [], [], [], []
    for wi, wj, horiz in adjacency:
        si, ei = ranges[wi]
        sj, ej = ranges[wj]
        for ci in range(si + 1, ei):
            for cj in range(sj + 1, ej):
                ci_l.append(ci)
                cj_l.append(cj)
                wi_l.append(wi)
                wj_l.append(wj)
                hz_l.append(horiz)

    target = np.arange(Ch)
    if not ci_l:
        return target

    sf = np.asarray(slot_features, np.float32)
    sf_n = sf / (np.linalg.norm(sf, axis=-1, keepdims=True) + np.float32(1e-8))
    ci_a = np.array(ci_l)
    cj_a = np.array(cj_l)
    rel_i = ci_a % mpw - 1
    rel_j = cj_a % mpw - 1
    fi = sf_n[np.array(wi_l), rel_i]
    fj = sf_n[np.array(wj_l), rel_j]
    sims = np.sum(fi * fj, axis=-1)
    hz = np.array(hz_l)
    edge_ok = np.where(hz, edge_r[ci_a] & edge_l[cj_a], edge_b[ci_a] & edge_t[cj_a])
    passing = edge_ok & (sims > np.float32(SIM_THRESH))

    merged = np.zeros(Ch, bool)
    for ci, cj, ok in zip(ci_l, cj_l, passing):
        if ok and not merged[ci] and not merged[cj]:
            keep, rem = min(ci, cj), max(ci, cj)
            target[target == rem] = keep
            merged[rem] = True
    return target


def _plan(remap):
    """Derive the device-op plan from remap.

    Returns (eff, keeps, groups) where eff is the sorted list of effective
    output channels (remap[c] == c; the rest are all-zero planes the host
    fills), keeps those eff channels that absorb merged channels, and
    groups[k] the full source list (k plus its merged channels).
    """
    eff = [c for c in range(C) if remap[c] == c]
    groups = {}
    for r in range(C):
        k = int(remap[r])
        if k != r:
            groups.setdefault(k, [k]).append(r)
    keeps = sorted(groups)
    return eff, keeps, groups


# --------------------------------------------------------------------------
# wait-split post-pass: the pinned neuronxcc allows only ONE sync wait per
# instruction; hoist extras onto preceding same-engine EventSemaphore insts.
# --------------------------------------------------------------------------
def _split_excess_waits(bir_json_bytes, limit=1):
    j = json.loads(bir_json_bytes)
    counter = [0]
    for fn in j.get("functions", []):
        for bb in fn.get("blocks", []):
            new_insts = []
            for inst in bb.get("instructions", []):
                si = inst.get("sync_info") or {}
                waits = si.get("on_wait") or []
                if len(waits) > limit:
                    extra = waits[: len(waits) - limit]
                    si["on_wait"] = waits[len(waits) - limit:]
                    inst["sync_info"] = si
                    for i in range(0, len(extra), limit):
                        counter[0] += 1
                        new_insts.append({
                            "engine": inst["engine"],
                            "ins": [],
                            "name": f"{inst['name']}_hoistw{counter[0]}",
                            "opcode": "EventSemaphore",
                            "outs": [],
                            "sync_info": {"on_update": [],
                                          "on_wait": extra[i: i + limit]},
                        })
                new_insts.append(inst)
            bb["instructions"] = new_insts
    return json.dumps(j).encode()


def _build_program(remap):
    key = tuple(int(v) for v in remap)
    if key in _cache:
        return _cache[key]

    import concourse.bass as bass
    import concourse.tile as tile
    from concourse import mybir

    f32 = mybir.dt.float32
    bf16 = mybir.dt.bfloat16
    eff, keeps, groups = _plan(remap)
    NE = len(eff)
    NK = len(keeps)
    kidx = {k: i for i, k in enumerate(keeps)}

    nc = bass.Bass()
    masks_in = nc.dram_tensor("masks", [C, ROWS_PER_CORE, IMG_W], f32,
                              kind="ExternalInput")
    out_dram = nc.dram_tensor("out", [NE, ROWS_PER_CORE, IMG_W], bf16,
                              kind="ExternalOutput")

    with tile.TileContext(nc) as tc:
        with (
            tc.tile_pool(name="inp", bufs=2) as inp,
            tc.tile_pool(name="outp", bufs=2) as outp,
            tc.tile_pool(name="tree", bufs=1) as tree,
            tc.tile_pool(name="small", bufs=2) as small,
        ):
            for t in range(NTILES):
                sl = slice(G * t, G * (t + 1))
                in_tile = inp.tile([128, C, G], f32, tag="in_tile")
                # channel-group split so the max-folds can start before the
                # whole tile lands
                for c0 in range(0, C, 8):
                    nc.sync.dma_start(
                        in_tile[:, c0:c0 + 8, :],
                        masks_in[c0:c0 + 8, :, sl].rearrange("d p g -> p d g"))

                # per-keep group maxes (merged channels fold into their keep)
                gs = tree.tile([128, NK + 1, G], f32, tag="gs")
                for k in keeps:
                    srcs = groups[k]
                    i = kidx[k]
                    nc.vector.tensor_tensor(
                        out=gs[:, i, :], in0=in_tile[:, srcs[0], :],
                        in1=in_tile[:, srcs[1], :], op=mybir.AluOpType.max)
                    for s in srcs[2:]:
                        nc.vector.tensor_tensor(
                            out=gs[:, i, :], in0=gs[:, i, :],
                            in1=in_tile[:, s, :], op=mybir.AluOpType.max)

                # global max = fold(plain effective channels, group maxes);
                # gs[0:NK] must stay intact for the one-hot compare below
                plains = [c for c in eff if c not in groups]
                items = [in_tile[:, c, :] for c in plains]
                items += [gs[:, i, :] for i in range(NK)]
                mx = small.tile([128, G], f32, tag="mx")
                acc = gs[:, NK, :]
                nc.vector.tensor_tensor(
                    out=acc, in0=items[0], in1=items[1],
                    op=mybir.AluOpType.max)
                for it in items[2:-1]:
                    nc.vector.tensor_tensor(
                        out=acc, in0=acc, in1=it, op=mybir.AluOpType.max)
                nc.vector.tensor_tensor(
                    out=mx[:], in0=acc, in1=items[-1],
                    op=mybir.AluOpType.max)

                # one-hot over effective channels only:
                # out[j] = (src[j] == mx), src = group max for keeps else in
                mx_ap = mx[:]
                out_tile = outp.tile([128, NE, G], bf16, tag="out_tile")
                runs = []  # (j0, src_tile_name, c0, n)
                for j, c in enumerate(eff):
                    if c in groups:
                        src, c0 = "gs", kidx[c]
                    else:
                        src, c0 = "in", c
                    if runs and runs[-1][1] == src and \
                            runs[-1][2] + runs[-1][3] == c0 and \
                            runs[-1][0] + runs[-1][3] == j:
                        runs[-1][3] += 1
                    else:
                        runs.append([j, src, c0, 1])
                for j0, src, c0, n in runs:
                    mx_b = bass.AP(tensor=mx_ap.tensor, offset=mx_ap.offset,
                                   ap=[mx_ap.ap[0], [0, n], mx_ap.ap[-1]])
                    src_ap = (gs if src == "gs" else in_tile)[:, c0:c0 + n, :]
                    nc.vector.tensor_tensor(
                        out=out_tile[:, j0:j0 + n, :], in0=src_ap, in1=mx_b,
                        op=mybir.AluOpType.is_equal)

                nc.scalar.dma_start(
                    out_dram[:, :, sl].rearrange("c p g -> p c g"), out_tile[:])

    orig = nc.to_json_bytes
    nc.to_json_bytes = lambda: _split_excess_waits(orig())
    _cache[key] = nc
    return nc


def kernel(masks, slot_features, pad_left, pad_top):
    from concourse.bass_utils import run_bass_kernel_spmd

    masks = np.asarray(masks, np.float32)
    slot_features = np.asarray(slot_features, np.float32)
    pl = [int(v) for v in np.asarray(pad_left)]
    pt = [int(v) for v in np.asarray(pad_top)]

    remap = _compute_remap(masks, slot_features, pl, pt)
    nc = _build_program(remap)

    in_maps = []
    for i in range(N_CORES):
        slab = np.ascontiguousarray(
            masks[0, :, i * ROWS_PER_CORE:(i + 1) * ROWS_PER_CORE, :])
        in_maps.append({"masks": slab})

    res = run_bass_kernel_spmd(nc, in_maps, core_ids=list(range(N_CORES)))

    eff, _, _ = _plan(remap)
    out = np.zeros((1, C, IMG_H, IMG_W), np.float32)
    for i, r in enumerate(res.results):
        out[0, eff, i * ROWS_PER_CORE:(i + 1) * ROWS_PER_CORE, :] = (
            np.asarray(r["out"]).astype(np.float32))

    # patch tied-max pixels (channel-sum != 1) from the input directly
    s = out[0].sum(axis=0)
    ys, xs = np.nonzero(s != 1.0)
    for y, x in zip(ys, xs):
        d = int(np.argmax(masks[0, :, y, x]))
        out[0, :, y, x] = 0.0
        out[0, remap[d], y, x] = 1.0
    return out


# revision 19
# speedup vs baseline: 7.0429x; 1.0804x over previous
"""nn_MergeWindows — Trainium2 Bass kernel (8 NeuronCores, SPMD over image rows).

The reference's output is out[b, c, y, x] = 1.0 iff remap[argmax_d masks[b, d,
y, x]] == c, where remap: [32]->[32] is the channel-merge map decided by the
sequential scan.  remap depends only on tiny metadata (boundary-strip argmaxes
+ slot-feature cosine sims) and is computed on the host in microseconds.

Device pipeline per [128 rows, 32 ch, G cols] tile, everything on the DVE:
  1. 5-level contiguous max-tree over the channel dim -> mx [128, G]
  2. win = is_equal(in, mx_broadcast)  -> one-hot over channels, bf16
  3. merge fixups: win[:, keep] += win[:, rem]; win[:, rem] = 0
     (a dozen tiny [128, G] ops, baked from remap)
  4. DMA the bf16 one-hot out; the host casts to f32 (0/1 exact in bf16).

Tied maxima (in[c] == in[d] == mx) make is_equal fire twice where argmax picks
the first channel.  Those pixels are exactly where the output's channel-sum is
not 1.0, so the host detects them from the assembled output and patches them
from the input directly (a handful of pixels on real data).
"""

import json

import numpy as np

N_WINDOWS = 4
WIN_H = WIN_W = 512
IMG_H = IMG_W = 1024
C = 32
MPW = C // N_WINDOWS
SLOT_DIM = 64
SIM_THRESH = 0.1

N_CORES = 8
ROWS_PER_CORE = IMG_H // N_CORES  # 128
G = 512                           # column-tile width
NTILES = IMG_W // G

_cache = {}


# --------------------------------------------------------------------------
# host-side merge decision (mirrors reference._merge_windows metadata math)
# --------------------------------------------------------------------------
def _compute_remap(masks, slot_features, pl, pt):
    B, Ch, H, W = masks.shape
    mpw = Ch // N_WINDOWS
    ranges = [(i * mpw, (i + 1) * mpw) for i in range(N_WINDOWS)]

    adjacency = []
    for i in range(N_WINDOWS):
        for j in range(i + 1, N_WINDOWS):
            if pt[i] == pt[j] and abs(pl[i] - pl[j]) == WIN_W:
                adjacency.append((i, j, True) if pl[i] < pl[j] else (j, i, True))
            if pl[i] == pl[j] and abs(pt[i] - pt[j]) == WIN_H:
                adjacency.append((i, j, False) if pt[i] < pt[j] else (j, i, False))

    edge_l = np.zeros(Ch, bool)
    edge_r = np.zeros(Ch, bool)
    edge_t = np.zeros(Ch, bool)
    edge_b = np.zeros(Ch, bool)
    m0 = masks[0]
    for wi, (s, e) in enumerate(ranges):
        ys, ye = max(pt[wi], 0), min(pt[wi] + WIN_H, H)
        xs, xe = max(pl[wi], 0), min(pl[wi] + WIN_W, W)
        if ys >= ye or xs >= xe:
            continue
        ids_l = np.argmax(m0[:, ys:ye, xs], axis=0)
        ids_r = np.argmax(m0[:, ys:ye, xe - 1], axis=0)
        ids_t = np.argmax(m0[:, ys, xs:xe], axis=0)
        ids_b = np.argmax(m0[:, ye - 1, xs:xe], axis=0)
        for k in range(s, e):
            edge_l[k] = np.any(ids_l == k)
            edge_r[k] = np.any(ids_r == k)
            edge_t[k] = np.any(ids_t == k)
            edge_b[k] = np.any(ids_b == k)

    ci_l, cj_l, wi_l, wj_l, hz_l = [], [], [], [], []
    for wi, wj, horiz in adjacency:
        si, ei = ranges[wi]
        sj, ej = ranges[wj]
        for ci in range(si + 1, ei):
            for cj in range(sj + 1, ej):
                ci_l.append(ci)
                cj_l.append(cj)
                wi_l.append(wi)
                wj_l.append(wj)
                hz_l.append(horiz)

    target = np.arange(Ch)
    if not ci_l:
        return target

    sf = np.asarray(slot_features, np.float32)
    sf_n = sf / (np.linalg.norm(sf, axis=-1, keepdims=True) + np.float32(1e-8))
    ci_a = np.array(ci_l)
    cj_a = np.array(cj_l)
    rel_i = ci_a % mpw - 1
    rel_j = cj_a % mpw - 1
    fi = sf_n[np.array(wi_l), rel_i]
    fj = sf_n[np.array(wj_l), rel_j]
    sims = np.sum(fi * fj, axis=-1)
    hz = np.array(hz_l)
    edge_ok = np.where(hz, edge_r[ci_a] & edge_l[cj_a], edge_b[ci_a] & edge_t[cj_a])
    passing = edge_ok & (sims > np.float32(SIM_THRESH))

    merged = np.zeros(Ch, bool)
    for ci, cj, ok in zip(ci_l, cj_l, passing):
        if ok and not merged[ci] and not merged[cj]:
            keep, rem = min(ci, cj), max(ci, cj)
            target[target == rem] = keep
            merged[rem] = True
    return target


def _plan(remap):
    """Derive the device-op plan from remap.

    Returns (eff, keeps, groups) where eff is the sorted list of effective
    output channels (remap[c] == c; the rest are all-zero planes the host
    fills), keeps those eff channels that absorb merged channels, and
    groups[k] the full source list (k plus its merged channels).
    """
    eff = [c for c in range(C) if remap[c] == c]
    groups = {}
    for r in range(C):
        k = int(remap[r])
        if k != r:
            groups.setdefault(k, [k]).append(r)
    keeps = sorted(groups)
    return eff, keeps, groups


# --------------------------------------------------------------------------
# wait-split post-pass: the pinned neuronxcc allows only ONE sync wait per
# instruction; hoist extras onto preceding same-engine EventSemaphore insts.
# --------------------------------------------------------------------------
def _split_excess_waits(bir_json_bytes, limit=1):
    j = json.loads(bir_json_bytes)
    counter = [0]
    for fn in j.get("functions", []):
        for bb in fn.get("blocks", []):
            new_insts = []
            for inst in bb.get("instructions", []):
                si = inst.get("sync_info") or {}
                waits = si.get("on_wait") or []
                if len(waits) > limit:
                    extra = waits[: len(waits) - limit]
                    si["on_wait"] = waits[len(waits) - limit:]
                    inst["sync_info"] = si
                    for i in range(0, len(extra), limit):
                        counter[0] += 1
                        new_insts.append({
                            "engine": inst["engine"],
                            "ins": [],
                            "name": f"{inst['name']}_hoistw{counter[0]}",
                            "opcode": "EventSemaphore",
                            "outs": [],
                            "sync_info": {"on_update": [],
                                          "on_wait": extra[i: i + limit]},
                        })
                new_insts.append(inst)
            bb["instructions"] = new_insts
    return json.dumps(j).encode()


def _build_program(remap):
    key = tuple(int(v) for v in remap)
    if key in _cache:
        return _cache[key]

    import concourse.bass as bass
    import concourse.tile as tile
    from concourse import mybir

    f32 = mybir.dt.float32
    bf16 = mybir.dt.bfloat16
    eff, keeps, groups = _plan(remap)
    NE = len(eff)
    NK = len(keeps)
    kidx = {k: i for i, k in enumerate(keeps)}

    nc = bass.Bass()
    masks_in = nc.dram_tensor("masks", [C, ROWS_PER_CORE, IMG_W], f32,
                              kind="ExternalInput")
    out_dram = nc.dram_tensor("out", [NE, ROWS_PER_CORE, IMG_W], bf16,
                              kind="ExternalOutput")

    with tile.TileContext(nc) as tc:
        with (
            tc.tile_pool(name="inp", bufs=2) as inp,
            tc.tile_pool(name="outp", bufs=2) as outp,
            tc.tile_pool(name="tree", bufs=2) as tree,
            tc.tile_pool(name="small", bufs=2) as small,
        ):
            plains = [c for c in eff if c not in groups]

            # one-hot source runs: (j0, src_tile_name, c0, n)
            runs = []
            for j, c in enumerate(eff):
                if c in groups:
                    src, c0 = "gs", kidx[c]
                else:
                    src, c0 = "in", c
                if runs and runs[-1][1] == src and \
                        runs[-1][2] + runs[-1][3] == c0 and \
                        runs[-1][0] + runs[-1][3] == j:
                    runs[-1][3] += 1
                else:
                    runs.append([j, src, c0, 1])
            # split runs into two out-DMA halves near NE/2 (runs align)
            hsplit = 0
            for j0, _, _, n in runs:
                if j0 + n <= (NE + 1) // 2:
                    hsplit = j0 + n
            runs_a = [r for r in runs if r[0] < hsplit]
            runs_b = [r for r in runs if r[0] >= hsplit]

            for t in range(NTILES):
                sl = slice(G * t, G * (t + 1))
                in_tile = inp.tile([128, C, G], f32, tag="in_tile")
                # channel-group split so the max-folds can start before the
                # whole tile lands; alternate dispatch queues
                step = 4 if t == 0 else 8
                for c0 in range(0, C, step):
                    nc.sync.dma_start(
                        in_tile[:, c0:c0 + step, :],
                        masks_in[c0:c0 + step, :, sl].rearrange(
                            "d p g -> p d g"))

                # global max via per-keep group maxes + plain folds, emitted
                # in channel-availability order.  gs[0:NK] stays intact for
                # the one-hot compare.
                gs = tree.tile([128, NK + 1, G], f32, tag="gs")
                acc = gs[:, NK, :]
                TT = nc.vector.tensor_tensor
                MAX = mybir.AluOpType.max
                prog = {k: 0 for k in keeps}
                acc_items = []   # APs not yet folded into acc
                emitted = 0

                def fold_into_acc():
                    nonlocal emitted
                    while acc_items and (len(acc_items) >= 2 or emitted):
                        if not emitted:
                            a, b = acc_items.pop(0), acc_items.pop(0)
                            TT(out=acc, in0=a, in1=b, op=MAX)
                        else:
                            TT(out=acc, in0=acc, in1=acc_items.pop(0), op=MAX)
                        emitted += 1

                for b in range(step, C + 1, step):
                    for k in keeps:
                        srcs, i = groups[k], kidx[k]
                        while prog[k] < len(srcs):
                            jn = prog[k]
                            if jn == 0:
                                if len(srcs) < 2 or srcs[1] >= b:
                                    break
                                TT(out=gs[:, i, :], in0=in_tile[:, srcs[0], :],
                                   in1=in_tile[:, srcs[1], :], op=MAX)
                                prog[k] = 2
                            elif srcs[jn] < b:
                                TT(out=gs[:, i, :], in0=gs[:, i, :],
                                   in1=in_tile[:, srcs[jn], :], op=MAX)
                                prog[k] += 1
                            else:
                                break
                        if prog[k] == len(srcs) and prog[k] > 0:
                            acc_items.append(gs[:, i, :])
                            prog[k] += 1  # mark folded
                    newly = [c for c in plains if c < b]
                    for c in newly:
                        acc_items.append(in_tile[:, c, :])
                        plains = [p for p in plains if p != c]
                    fold_into_acc()
                plains = [c for c in eff if c not in groups]  # reset for next t

                # one-hot over effective channels; two halves, each DMA'd as
                # soon as its runs are done.  The global max lives in acc
                # (gs[:, NK, :]); mx tile is unused.
                mx_ap = acc
                out_tile = outp.tile([128, NE, G], bf16, tag="out_tile")

                def emit_half(hruns, j0, j1):
                    for r0, src, c0, n in hruns:
                        mx_b = bass.AP(
                            tensor=mx_ap.tensor, offset=mx_ap.offset,
                            ap=[mx_ap.ap[0], [0, n], mx_ap.ap[-1]])
                        src_ap = (gs if src == "gs" else in_tile)[:, c0:c0 + n, :]
                        nc.vector.tensor_tensor(
                            out=out_tile[:, r0:r0 + n, :], in0=src_ap,
                            in1=mx_b, op=mybir.AluOpType.is_equal)
                    nc.scalar.dma_start(
                        out_dram[j0:j1, :, sl].rearrange("c p g -> p c g"),
                        out_tile[:, j0:j1, :])

                emit_half(runs_a, 0, hsplit)
                emit_half(runs_b, hsplit, NE)

    orig = nc.to_json_bytes
    nc.to_json_bytes = lambda: _split_excess_waits(orig())
    _cache[key] = nc
    return nc


def kernel(masks, slot_features, pad_left, pad_top):
    from concourse.bass_utils import run_bass_kernel_spmd

    masks = np.asarray(masks, np.float32)
    slot_features = np.asarray(slot_features, np.float32)
    pl = [int(v) for v in np.asarray(pad_left)]
    pt = [int(v) for v in np.asarray(pad_top)]

    remap = _compute_remap(masks, slot_features, pl, pt)
    nc = _build_program(remap)

    in_maps = []
    for i in range(N_CORES):
        slab = np.ascontiguousarray(
            masks[0, :, i * ROWS_PER_CORE:(i + 1) * ROWS_PER_CORE, :])
        in_maps.append({"masks": slab})

    res = run_bass_kernel_spmd(nc, in_maps, core_ids=list(range(N_CORES)))

    eff, _, _ = _plan(remap)
    out = np.zeros((1, C, IMG_H, IMG_W), np.float32)
    for i, r in enumerate(res.results):
        out[0, eff, i * ROWS_PER_CORE:(i + 1) * ROWS_PER_CORE, :] = (
            np.asarray(r["out"]).astype(np.float32))

    # patch tied-max pixels (channel-sum != 1) from the input directly
    s = out[0].sum(axis=0)
    ys, xs = np.nonzero(s != 1.0)
    for y, x in zip(ys, xs):
        d = int(np.argmax(masks[0, :, y, x]))
        out[0, :, y, x] = 0.0
        out[0, remap[d], y, x] = 1.0
    return out
